# revision 16
# baseline (speedup 1.0000x reference)
"""Trainium2 Bass kernel for gated inception-conv attention (8 cores, seq-parallel).

Shapes (hardcoded): q_data/k_data (1,8,1024,512) f32, bias (1,8,1024,1024) f32,
k_mask (1,8,1024) i32, Wq/Wk/Wv/Wg (512,512), bg (512), Wo (512,512), bo (512),
qcw/kcw/vcw (64,1,3), qcb/kcb/vcb (64).  Output (1,8,1024,512) f32.

Strategy (v3): one sequence per NeuronCore, all-bf16 compute, f32 PSUM.
 - Conv phase as in v1: projections on PE (psum -> padded sbuf via ACT),
   depthwise conv as 5 shifted DVE ops, v transposed to natural layout on PE.
 - Scores transposed [kpos, q]; exp on ACT (k-mask via bias port, D^-0.5 via
   scale port); exp(bias)^T multiplied in on GpSimd.
 - AV flipped vs v1: score chunks stationary [128kpos x 128q], v natural
   [128kpos x 65] (ones column -> softmax denominator) is the moving operand;
   output lands NATURAL [q, d], so normalization is a per-partition
   tensor_scalar riding the psum-drain copy (reciprocal_approx_fast).
 - Gate computed in natural layout (bg via a K=1 ones-row matmul), gating on
   GpSimd; PE transposes og for the final projection; out = natural [q, c]
   (+bo via DVE add of a host-broadcast tile), DMA'd per q-chunk.
"""

import os
import sys

sys.path.insert(0, "/opt/trn_rl_repo")

import numpy as np
import ml_dtypes

import concourse.bass as bass
import concourse.mybir as mybir
from concourse import bacc, tile
from concourse.bass_utils import run_bass_kernel_spmd

BF16 = ml_dtypes.bfloat16
F32 = mybir.dt.float32
BF16D = mybir.dt.bfloat16
FP = mybir.ActivationFunctionType
MULT = mybir.AluOpType.mult
ADD = mybir.AluOpType.add

H, D, L, C = 8, 64, 1024, 512
KD = VD = 512
NCORES = 8
MASK_NEG = -100000.0
SCALE = 0.125  # D**-0.5, applied at the exp

ONESMM = os.environ.get("K_ONESMM", "1") == "1"
GPS = os.environ.get("K_GPS", "1") == "1"


def build():
    nc = bacc.Bacc(
        "TRN2",
        target_bir_lowering=False,
        debug=False,
        enable_asserts=False,
    )

    # ---- DRAM I/O (host pre-laid-out) ----
    qd = nc.dram_tensor("qd", [128, 4, L], BF16D, kind="ExternalInput").ap()
    kd = nc.dram_tensor("kd", [128, 4, L], BF16D, kind="ExternalInput").ap()
    # exp(bias)^T, bf16, fused head pairs: [hp, kc, p, h01*1024 + q]
    ebias = nc.dram_tensor("ebias", [4, 8, 128, 2048], BF16D, kind="ExternalInput").ap()
    maskneg = nc.dram_tensor("maskneg", [128, 8], F32, kind="ExternalInput").ap()
    wq = nc.dram_tensor("wq", [128, 4, KD], BF16D, kind="ExternalInput").ap()
    wk = nc.dram_tensor("wk", [128, 4, KD], BF16D, kind="ExternalInput").ap()
    wv = nc.dram_tensor("wv", [128, 4, VD], BF16D, kind="ExternalInput").ap()
    wg = nc.dram_tensor("wg", [128, 4, VD], BF16D, kind="ExternalInput").ap()
    wo = nc.dram_tensor("wo", [128, 4, C], BF16D, kind="ExternalInput").ap()
    convw = nc.dram_tensor("convw", [128, 12], F32, kind="ExternalInput").ap()
    bgn = nc.dram_tensor("bgn", [1, VD], BF16D, kind="ExternalInput").ap()
    bob = nc.dram_tensor("bob", [128, C], F32, kind="ExternalInput").ap()
    identv = nc.dram_tensor("identv", [128, 64], BF16D, kind="ExternalInput").ap()
    idento = nc.dram_tensor("idento", [128, 128], BF16D, kind="ExternalInput").ap()
    out = nc.dram_tensor("out", [8, 128, C], F32, kind="ExternalOutput").ap()

    with tile.TileContext(nc) as tc, nc.allow_low_precision(
        reason="bf16 compute; rel-err budget 2e-2"
    ):
        _body(tc, locals())
    nc.compile()
    return nc


def _body(tc, t):
    nc = tc.nc
    qd, kd, ebias, maskneg = t["qd"], t["kd"], t["ebias"], t["maskneg"]
    wq, wk, wv, wg, wo = t["wq"], t["wk"], t["wv"], t["wg"], t["wo"]
    convw, bgn, bob, identv, idento, out = (
        t["convw"], t["bgn"], t["bob"], t["identv"], t["idento"], t["out"],
    )

    with tc.tile_pool(name="const", bufs=1) as const, \
         tc.tile_pool(name="big", bufs=1) as big, \
         tc.tile_pool(name="ebp", bufs=4) as ebp, \
         tc.tile_pool(name="inw", bufs=1) as inw, \
         tc.tile_pool(name="recp", bufs=2) as recp, \
         tc.tile_pool(name="stage", bufs=2) as stg, \
         tc.tile_pool(name="qkps", bufs=2, space="PSUM") as qkp, \
         tc.tile_pool(name="avps", bufs=2, space="PSUM") as avp:

        # ---- inputs needed first: activations + projection weights ----
        qdT = inw.tile([128, 4, L], BF16D, name="qdT", tag="qdT")
        for ks in range(4):
            for h2 in range(2):
                nc.sync.dma_start(
                    qdT[:, ks, h2 * 512 : (h2 + 1) * 512],
                    qd[:, ks, h2 * 512 : (h2 + 1) * 512],
                )
        w_sb = {}
        for nm, wd in (("q", wq), ("k", wk), ("v", wv), ("g", wg)):
            w_sb[nm] = inw.tile([128, 4, 512], BF16D, name=f"w{nm}", tag=f"w{nm}")
        for ks in range(4):
            nc.sync.dma_start(w_sb["q"][:, ks, :], wq[:, ks, :])
        kdT = inw.tile([128, 4, L], BF16D, name="kdT", tag="kdT")
        for ks in range(4):
            for h2 in range(2):
                nc.sync.dma_start(
                    kdT[:, ks, h2 * 512 : (h2 + 1) * 512],
                    kd[:, ks, h2 * 512 : (h2 + 1) * 512],
                )
        for nm, wd in (("k", wk), ("v", wv), ("g", wg)):
            for ks in range(4):
                nc.sync.dma_start(w_sb[nm][:, ks, :], wd[:, ks, :])

        # ---- other constants ----
        convw_sb = const.tile([128, 12], F32, name="convw", tag="convw")
        nc.sync.dma_start(convw_sb[:], convw)
        identv_sb = const.tile([128, 64], BF16D, name="identv", tag="identv")
        nc.sync.dma_start(identv_sb[:], identv)
        idento_sb = const.tile([128, 128], BF16D, name="idento", tag="idento")
        nc.sync.dma_start(idento_sb[:], idento)
        maskneg_sb = const.tile([128, 8], F32, name="maskneg", tag="maskneg")
        nc.sync.dma_start(maskneg_sb[:], maskneg)
        bgn_sb = const.tile([1, VD], BF16D, name="bgn", tag="bgn")
        nc.sync.dma_start(bgn_sb[:], bgn)
        bob_sb = const.tile([128, C], F32, name="bob", tag="bob")
        nc.sync.dma_start(bob_sb[:], bob)
        wo_sb = const.tile([128, 4, C], BF16D, name="wo", tag="wo")
        nc.sync.dma_start(wo_sb[:], wo)
        ones_sb = const.tile([1, 128], BF16D, name="ones", tag="ones")
        nc.vector.memset(ones_sb[:], 1.0)

        # ---- persistent big tensors ----
        qc_t = [big.tile([128, L], BF16D, name=f"qc{c}", tag=f"qc{c}") for c in range(4)]
        kc_t = [big.tile([128, L], BF16D, name=f"kc{c}", tag=f"kc{c}") for c in range(4)]
        vnat = big.tile([128, H, 8, D + 1], BF16D, name="vnat", tag="vnat")
        nc.vector.memset(vnat[:, :, :, D : D + 1], 1.0)
        gN = big.tile([128, 8, VD], BF16D, name="gN", tag="gN")
        oN = big.tile([128, 8, H, D], BF16D, name="oN", tag="oN")
        eT = [
            [
                big.tile([128, 8, L], BF16D, name=f"eT{par}{h01}",
                         tag=f"eT{par}{h01}")
                for h01 in range(2)
            ]
            for par in range(2)
        ]
        ogT = big.tile([128, 4, L], BF16D, name="ogT", tag="ogT")

        # ====== conv phase (v1 structure): projections -> conv -> v-T ======
        rhsT = {"q": qdT, "k": kdT, "v": kdT}
        cw = {"q": 0, "k": 4, "v": 8}
        with tc.tile_pool(name="pad", bufs=2) as padp, \
             tc.tile_pool(name="cvtmp", bufs=2) as cvp, \
             tc.tile_pool(name="vc", bufs=2) as vcp:
            for c in range(4):
                pads = {}
                for nm in ("q", "k", "v"):
                    pads[nm] = padp.tile(
                        [128, L + 2], BF16D, name=f"pad{nm}", tag=f"pad{nm}"
                    )
                    nc.vector.memset(pads[nm][:, 0:1], 0.0)
                    nc.vector.memset(pads[nm][:, L + 1 : L + 2], 0.0)
                    for q2 in range(2):
                        ps = qkp.tile([128, 512], F32, name="projps", tag="qk")
                        for ks in range(4):
                            nc.tensor.matmul(
                                ps[:],
                                w_sb[nm][:, ks, c * 128 : (c + 1) * 128],
                                rhsT[nm][:, ks, q2 * 512 : (q2 + 1) * 512],
                                start=(ks == 0),
                                stop=(ks == 3),
                            )
                        nc.scalar.copy(
                            pads[nm][:, 1 + q2 * 512 : 1 + (q2 + 1) * 512], ps[:]
                        )
                vc = vcp.tile([128, L], BF16D, name="vc", tag="vc")
                dstc = {"q": qc_t[c], "k": kc_t[c], "v": vc}
                for nm in ("q", "k", "v"):
                    base = cw[nm]
                    w0 = convw_sb[:, base : base + 1]
                    w1 = convw_sb[:, base + 1 : base + 2]
                    w2 = convw_sb[:, base + 2 : base + 3]
                    bb = convw_sb[:, base + 3 : base + 4]
                    x = pads[nm]
                    y = dstc[nm]
                    nc.vector.tensor_scalar(y[:], x[:, 1 : L + 1], w1, bb, MULT, ADD)
                    tm = cvp.tile([128, L], BF16D, name="cvtmp", tag="cvtmp")
                    nc.vector.tensor_scalar_mul(tm[:], x[:, 0:L], w0)
                    nc.vector.tensor_tensor(y[:], y[:], tm[:], ADD)
                    tm2 = cvp.tile([128, L], BF16D, name="cvtmp2", tag="cvtmp")
                    nc.vector.tensor_scalar_mul(tm2[:], x[:, 2 : L + 2], w2)
                    nc.vector.tensor_tensor(y[:], y[:], tm2[:], ADD)
                # v -> natural layout, heads 2c/2c+1 row-packed on the PE
                pss = [
                    avp.tile([128, 512], BF16D, name="vtps", tag="av")
                    for _ in range(2)
                ]
                for b in range(8):
                    for h01 in range(2):
                        ph = h01 * 64
                        nc.tensor.transpose(
                            pss[h01][:, b * 64 : (b + 1) * 64],
                            vc[ph : ph + 64, b * 128 : (b + 1) * 128],
                            identv_sb[ph : ph + 64, :],
                        )
                for h01 in range(2):
                    nc.vector.tensor_copy(
                        vnat[:, 2 * c + h01, :, 0:D],
                        pss[h01].rearrange("p (a b) -> p a b", b=64),
                    )

        # ====== gate projection, natural layout ============================
        for qc in range(8):
            gps = qkp.tile([128, 512], F32, name="gps", tag="qk")
            for ks in range(4):
                nc.tensor.matmul(
                    gps[:],
                    qdT[:, ks, qc * 128 : (qc + 1) * 128],
                    w_sb["g"][:, ks, :],
                    start=(ks == 0),
                    stop=(False if ONESMM else ks == 3),
                )
            if ONESMM:
                nc.tensor.matmul(
                    gps[:], ones_sb[:], bgn_sb[:], start=False, stop=True,
                )
            nc.scalar.activation(gN[:, qc, :], gps[:], FP.Sigmoid)

        # ====== attention ==================================================
        def av_block(hp, qcs):
            eTv = eT[hp % 2]
            for qc in qcs:
                av = avp.tile([128, 2, 66], F32, name="av", tag="av")
                for h01 in range(2):
                    for kc in range(8):
                        nc.tensor.matmul(
                            av[:, h01, 0 : D + 1],
                            eTv[h01][:, kc, qc * 128 : (qc + 1) * 128],
                            vnat[:, 2 * hp + h01, kc, :],
                            start=(kc == 0),
                            stop=(kc == 7),
                        )
                rec = recp.tile([128, 2], F32, name="rec", tag="rec")
                for h01 in range(2):
                    nc.vector.reciprocal_approx_fast(
                        rec[:, h01 : h01 + 1], av[:, h01, D : D + 1]
                    )
                for h01 in range(2):
                    nc.vector.tensor_scalar_mul(
                        oN[:, qc, 2 * hp + h01, :],
                        av[:, h01, 0:D],
                        rec[:, h01 : h01 + 1],
                    )
                (nc.gpsimd if GPS else nc.vector).tensor_tensor(
                    oN[:, qc, 2 * hp : 2 * hp + 2, :],
                    oN[:, qc, 2 * hp : 2 * hp + 2, :],
                    gN[:, qc, 2 * hp * 64 : (2 * hp + 2) * 64].rearrange(
                        "p (b c) -> p b c", b=2
                    ),
                    MULT,
                )

        for hp in range(4):
            eTp = eT[hp % 2]
            for j in range(4):
                ebs = []
                for h01 in range(2):
                    eb = ebp.tile([128, 2, L], BF16D, name="ebst", tag="ebst")
                    nc.sync.dma_start(
                        eb[:],
                        ebias[
                            hp, 2 * j : 2 * j + 2, :,
                            h01 * 1024 : (h01 + 1) * 1024,
                        ].rearrange("kc p q -> p kc q"),
                    )
                    ebs.append(eb)
                for kk in range(2):
                    kc = 2 * j + kk
                    for h01 in range(2):
                        ph = h01 * 64
                        qk = qkp.tile([128, L], F32, name="qk", tag="qk")
                        for q2 in range(2):
                            nc.tensor.matmul(
                                qk[:, q2 * 512 : (q2 + 1) * 512],
                                kc_t[hp][ph : ph + 64, kc * 128 : (kc + 1) * 128],
                                qc_t[hp][ph : ph + 64, q2 * 512 : (q2 + 1) * 512],
                                start=True,
                                stop=True,
                            )
                        nc.scalar.activation(
                            eTp[h01][:, kc, :], qk[:], FP.Exp,
                            bias=maskneg_sb[:, kc : kc + 1], scale=SCALE,
                        )
                for h01 in range(2):
                    ee = nc.gpsimd if (GPS and (j + h01) % 2 == 0 and hp % 2 == 0) \
                        else nc.vector
                    ee.tensor_tensor(
                        eTp[h01][:, 2 * j : 2 * j + 2, :],
                        eTp[h01][:, 2 * j : 2 * j + 2, :],
                        ebs[h01][:],
                        MULT,
                    )
            av_block(hp, range(8))

        # ====== epilogue: transpose og, out-projection =====================
        for qc in range(8):
            tp = qkp.tile([128, 4, 128], BF16D, name="ogtp", tag="qk")
            for vdc in range(4):
                nc.tensor.transpose(
                    tp[:, vdc, :],
                    oN[:, qc, 2 * vdc : 2 * vdc + 2, :].rearrange(
                        "p a b -> p (a b)"
                    ),
                    idento_sb[:],
                )
            nc.vector.tensor_copy(ogT[:, :, qc * 128 : (qc + 1) * 128], tp[:])
        for qc in range(8):
            pso = avp.tile([128, 512], F32, name="ops", tag="av")
            for vdc in range(4):
                nc.tensor.matmul(
                    pso[:],
                    ogT[:, vdc, qc * 128 : (qc + 1) * 128],
                    wo_sb[:, vdc, :],
                    start=(vdc == 0),
                    stop=(vdc == 3),
                )
            ost = stg.tile([128, C], F32, name="ost", tag="ost")
            nc.vector.tensor_tensor(ost[:], pso[:], bob_sb[:], ADD)
            for q4 in range(4):
                nc.sync.dma_start(
                    out[qc][:, q4 * 128 : (q4 + 1) * 128],
                    ost[:, q4 * 128 : (q4 + 1) * 128],
                )


# ---------------------------------------------------------------------------
# host side
# ---------------------------------------------------------------------------
_NC = None


def _get_nc():
    global _NC
    if _NC is None:
        _NC = build()
    return _NC


def _chunked(w):
    """(512, N) -> (128, 4, N) with row r at [r % 128, r // 128]."""
    n = np.asarray(w).shape[1]
    return np.ascontiguousarray(
        np.asarray(w, np.float32).reshape(4, 128, n).transpose(1, 0, 2)
    ).astype(BF16)


def _ctrans(x):
    """(L, C) -> (128, 4, L) bf16 with channel r at [r % 128, r // 128]."""
    xT = np.asarray(x, np.float32).T  # (C, L)
    return np.ascontiguousarray(
        xT.reshape(4, 128, L).transpose(1, 0, 2)
    ).astype(BF16)


def _prep_inmaps(q_data, k_data, bias, k_mask, Wq, Wk, Wv, Wg, bg, Wo, bo,
                 qcw, qcb, kcw, kcb, vcw, vcb):
    f32 = np.float32
    # exp(bias)^T -> (4, 8, 128, 2048) bf16: [hp, kc, p, h01*1024 + q]
    ebT = np.exp(np.asarray(bias[0], f32)).transpose(0, 2, 1)  # (h, k, q)
    ebias = np.ascontiguousarray(
        ebT.reshape(4, 2, 8, 128, L).transpose(0, 2, 3, 1, 4).reshape(4, 8, 128, 2 * L)
    ).astype(BF16)

    wq_a, wk_a, wv_a, wg_a, wo_a = (_chunked(w) for w in (Wq, Wk, Wv, Wg, Wo))
    bgn = np.asarray(bg, f32).reshape(1, VD).astype(BF16)
    bob = np.ascontiguousarray(
        np.broadcast_to(np.asarray(bo, f32).reshape(1, C), (128, C))
    ).astype(f32)

    identv = np.zeros((128, 64), f32)
    identv[np.arange(128), np.arange(128) % 64] = 1.0
    identv = identv.astype(BF16)
    idento = np.eye(128, dtype=f32).astype(BF16)

    dd = np.arange(128) % 64

    def taps(w3, b1, use_real):
        cwc = np.zeros((128, 4), f32)
        if use_real:
            cwc[:, 0:3] = np.asarray(w3, f32)[dd, 0, :]
            cwc[:, 3] = np.asarray(b1, f32)[dd]
        else:
            cwc[:, 1] = 1.0
        return cwc

    in_maps = []
    for s in range(NCORES):
        real = s >= 4
        cwm = np.concatenate(
            [taps(qcw, qcb, real), taps(kcw, kcb, real), taps(vcw, vcb, real)],
            axis=1,
        ).astype(f32)
        mk = np.asarray(k_mask[0, s], np.int32).reshape(8, 128).T  # (128, 8)
        maskneg = np.where(mk != 0, 0.0, MASK_NEG).astype(f32)
        in_maps.append(
            {
                "qd": _ctrans(q_data[0, s]),
                "kd": _ctrans(k_data[0, s]),
                "ebias": ebias,
                "maskneg": maskneg,
                "wq": wq_a, "wk": wk_a, "wv": wv_a, "wg": wg_a, "wo": wo_a,
                "convw": cwm,
                "bgn": bgn,
                "bob": bob,
                "identv": identv,
                "idento": idento,
            }
        )
    return in_maps


def run(in_maps, trace=False):
    nc = _get_nc()
    return run_bass_kernel_spmd(
        nc, in_maps, core_ids=list(range(NCORES)), trace=trace
    )


def kernel(**inputs):
    in_maps = _prep_inmaps(**inputs)
    res = run(in_maps)
    outp = np.empty((1, NCORES, L, C), np.float32)
    for s in range(NCORES):
        outp[0, s] = res.results[s]["out"].reshape(L, C)
    return outp


# revision 17
# speedup vs baseline: 1.1411x; 1.1411x over previous
"""Trainium2 Bass kernel for gated inception-conv attention (8 cores, seq-parallel).

Shapes (hardcoded): q_data/k_data (1,8,1024,512) f32, bias (1,8,1024,1024) f32,
k_mask (1,8,1024) i32, Wq/Wk/Wv/Wg (512,512), bg (512), Wo (512,512), bo (512),
qcw/kcw/vcw (64,1,3), qcb/kcb/vcb (64).  Output (1,8,1024,512) f32.

Strategy (v3): one sequence per NeuronCore, all-bf16 compute, f32 PSUM.
 - Conv phase as in v1: projections on PE (psum -> padded sbuf via ACT),
   depthwise conv as 5 shifted DVE ops, v transposed to natural layout on PE.
 - Scores transposed [kpos, q]; exp on ACT (k-mask via bias port, D^-0.5 via
   scale port); exp(bias)^T multiplied in on GpSimd.
 - AV flipped vs v1: score chunks stationary [128kpos x 128q], v natural
   [128kpos x 65] (ones column -> softmax denominator) is the moving operand;
   output lands NATURAL [q, d], so normalization is a per-partition
   tensor_scalar riding the psum-drain copy (reciprocal_approx_fast).
 - Gate computed in natural layout (bg via a K=1 ones-row matmul), gating on
   GpSimd; PE transposes og for the final projection; out = natural [q, c]
   (+bo via DVE add of a host-broadcast tile), DMA'd per q-chunk.
"""

import os
import sys

sys.path.insert(0, "/opt/trn_rl_repo")

import numpy as np
import ml_dtypes

import concourse.bass as bass
import concourse.mybir as mybir
from concourse import bacc, tile
from concourse.bass_utils import run_bass_kernel_spmd

BF16 = ml_dtypes.bfloat16
F32 = mybir.dt.float32
BF16D = mybir.dt.bfloat16
FP = mybir.ActivationFunctionType
MULT = mybir.AluOpType.mult
ADD = mybir.AluOpType.add

H, D, L, C = 8, 64, 1024, 512
KD = VD = 512
NCORES = 8
MASK_NEG = -100000.0
SCALE = 0.125  # D**-0.5, applied at the exp

ONESMM = os.environ.get("K_ONESMM", "1") == "1"
GPS = os.environ.get("K_GPS", "1") == "1"


def build():
    nc = bacc.Bacc(
        "TRN2",
        target_bir_lowering=False,
        debug=False,
        enable_asserts=False,
    )

    # ---- DRAM I/O (host pre-laid-out) ----
    qd = nc.dram_tensor("qd", [128, 4, L], BF16D, kind="ExternalInput").ap()
    kd = nc.dram_tensor("kd", [128, 4, L], BF16D, kind="ExternalInput").ap()
    # exp(bias)^T, bf16, fused head pairs: [hp, kc, p, h01*1024 + q]
    ebias = nc.dram_tensor("ebias", [4, 8, 128, 2048], BF16D, kind="ExternalInput").ap()
    maskneg = nc.dram_tensor("maskneg", [128, 8], F32, kind="ExternalInput").ap()
    wq = nc.dram_tensor("wq", [128, 4, KD], BF16D, kind="ExternalInput").ap()
    wk = nc.dram_tensor("wk", [128, 4, KD], BF16D, kind="ExternalInput").ap()
    wv = nc.dram_tensor("wv", [128, 4, VD], BF16D, kind="ExternalInput").ap()
    wg = nc.dram_tensor("wg", [128, 4, VD], BF16D, kind="ExternalInput").ap()
    wo = nc.dram_tensor("wo", [128, 4, C], BF16D, kind="ExternalInput").ap()
    convw = nc.dram_tensor("convw", [128, 12], F32, kind="ExternalInput").ap()
    bgn = nc.dram_tensor("bgn", [1, VD], BF16D, kind="ExternalInput").ap()
    bob = nc.dram_tensor("bob", [128, C], F32, kind="ExternalInput").ap()
    identv = nc.dram_tensor("identv", [128, 64], BF16D, kind="ExternalInput").ap()
    idento = nc.dram_tensor("idento", [128, 128], BF16D, kind="ExternalInput").ap()
    out = nc.dram_tensor("out", [8, 128, C], F32, kind="ExternalOutput").ap()

    with tile.TileContext(nc) as tc, nc.allow_low_precision(
        reason="bf16 compute; rel-err budget 2e-2"
    ):
        _body(tc, locals())
    nc.compile()
    return nc


def _body(tc, t):
    nc = tc.nc
    qd, kd, ebias, maskneg = t["qd"], t["kd"], t["ebias"], t["maskneg"]
    wq, wk, wv, wg, wo = t["wq"], t["wk"], t["wv"], t["wg"], t["wo"]
    convw, bgn, bob, identv, idento, out = (
        t["convw"], t["bgn"], t["bob"], t["identv"], t["idento"], t["out"],
    )

    with tc.tile_pool(name="const", bufs=1) as const, \
         tc.tile_pool(name="big", bufs=1) as big, \
         tc.tile_pool(name="ebp", bufs=4) as ebp, \
         tc.tile_pool(name="inw", bufs=1) as inw, \
         tc.tile_pool(name="recp", bufs=2) as recp, \
         tc.tile_pool(name="stage", bufs=2) as stg, \
         tc.tile_pool(name="qkps", bufs=2, space="PSUM") as qkp, \
         tc.tile_pool(name="avps", bufs=2, space="PSUM") as avp:

        # ---- inputs needed first: activations + projection weights ----
        qdT = inw.tile([128, 4, L], BF16D, name="qdT", tag="qdT")
        for ks in range(4):
            nc.sync.dma_start(qdT[:, ks, :], qd[:, ks, :])
        w_sb = {}
        for nm, wd in (("q", wq), ("k", wk), ("v", wv), ("g", wg)):
            w_sb[nm] = inw.tile([128, 4, 512], BF16D, name=f"w{nm}", tag=f"w{nm}")
        nc.sync.dma_start(w_sb["q"][:], wq)
        kdT = inw.tile([128, 4, L], BF16D, name="kdT", tag="kdT")
        for ks in range(4):
            nc.sync.dma_start(kdT[:, ks, :], kd[:, ks, :])
        nc.sync.dma_start(w_sb["k"][:], wk)
        nc.sync.dma_start(w_sb["v"][:], wv)
        nc.sync.dma_start(w_sb["g"][:], wg)

        # ---- other constants ----
        convw_sb = const.tile([128, 12], F32, name="convw", tag="convw")
        nc.sync.dma_start(convw_sb[:], convw)
        identv_sb = const.tile([128, 64], BF16D, name="identv", tag="identv")
        nc.sync.dma_start(identv_sb[:], identv)
        idento_sb = const.tile([128, 128], BF16D, name="idento", tag="idento")
        nc.sync.dma_start(idento_sb[:], idento)
        maskneg_sb = const.tile([128, 8], F32, name="maskneg", tag="maskneg")
        nc.sync.dma_start(maskneg_sb[:], maskneg)
        bgn_sb = const.tile([1, VD], BF16D, name="bgn", tag="bgn")
        nc.sync.dma_start(bgn_sb[:], bgn)
        bob_sb = const.tile([128, C], F32, name="bob", tag="bob")
        nc.sync.dma_start(bob_sb[:], bob)
        wo_sb = const.tile([128, 4, C], BF16D, name="wo", tag="wo")
        nc.sync.dma_start(wo_sb[:], wo)
        ones_sb = const.tile([1, 128], BF16D, name="ones", tag="ones")
        nc.vector.memset(ones_sb[:], 1.0)

        # ---- persistent big tensors ----
        qc_t = [big.tile([128, L], BF16D, name=f"qc{c}", tag=f"qc{c}") for c in range(4)]
        kc_t = [big.tile([128, L], BF16D, name=f"kc{c}", tag=f"kc{c}") for c in range(4)]
        vnat = big.tile([128, H, 8, D + 1], BF16D, name="vnat", tag="vnat")
        nc.vector.memset(vnat[:, :, :, D : D + 1], 1.0)
        gN = big.tile([128, 8, VD], BF16D, name="gN", tag="gN")
        oN = big.tile([128, 8, H, D], BF16D, name="oN", tag="oN")
        eT = [
            [
                big.tile([128, 8, L], BF16D, name=f"eT{par}{h01}",
                         tag=f"eT{par}{h01}")
                for h01 in range(2)
            ]
            for par in range(2)
        ]
        ogT = big.tile([128, 4, L], BF16D, name="ogT", tag="ogT")

        # ====== conv phase (v1 structure): projections -> conv -> v-T ======
        rhsT = {"q": qdT, "k": kdT, "v": kdT}
        cw = {"q": 0, "k": 4, "v": 8}
        with tc.tile_pool(name="pad", bufs=2) as padp, \
             tc.tile_pool(name="cvtmp", bufs=2) as cvp, \
             tc.tile_pool(name="vc", bufs=2) as vcp:
            for c in range(4):
                pads = {}
                for nm in ("q", "k", "v"):
                    pads[nm] = padp.tile(
                        [128, L + 2], BF16D, name=f"pad{nm}", tag=f"pad{nm}"
                    )
                    nc.vector.memset(pads[nm][:, 0:1], 0.0)
                    nc.vector.memset(pads[nm][:, L + 1 : L + 2], 0.0)
                    for q2 in range(2):
                        ps = qkp.tile([128, 512], F32, name="projps", tag="qk")
                        for ks in range(4):
                            nc.tensor.matmul(
                                ps[:],
                                w_sb[nm][:, ks, c * 128 : (c + 1) * 128],
                                rhsT[nm][:, ks, q2 * 512 : (q2 + 1) * 512],
                                start=(ks == 0),
                                stop=(ks == 3),
                            )
                        nc.scalar.copy(
                            pads[nm][:, 1 + q2 * 512 : 1 + (q2 + 1) * 512], ps[:]
                        )
                vc = vcp.tile([128, L], BF16D, name="vc", tag="vc")
                dstc = {"q": qc_t[c], "k": kc_t[c], "v": vc}
                for nm in ("q", "k", "v"):
                    base = cw[nm]
                    w0 = convw_sb[:, base : base + 1]
                    w1 = convw_sb[:, base + 1 : base + 2]
                    w2 = convw_sb[:, base + 2 : base + 3]
                    bb = convw_sb[:, base + 3 : base + 4]
                    x = pads[nm]
                    y = dstc[nm]
                    nc.vector.tensor_scalar(y[:], x[:, 1 : L + 1], w1, bb, MULT, ADD)
                    tm = cvp.tile([128, L], BF16D, name="cvtmp", tag="cvtmp")
                    nc.vector.tensor_scalar_mul(tm[:], x[:, 0:L], w0)
                    nc.vector.tensor_tensor(y[:], y[:], tm[:], ADD)
                    tm2 = cvp.tile([128, L], BF16D, name="cvtmp2", tag="cvtmp")
                    nc.vector.tensor_scalar_mul(tm2[:], x[:, 2 : L + 2], w2)
                    nc.vector.tensor_tensor(y[:], y[:], tm2[:], ADD)
                # v -> natural layout, heads 2c/2c+1 row-packed on the PE
                pss = [
                    avp.tile([128, 512], BF16D, name="vtps", tag="av")
                    for _ in range(2)
                ]
                for b in range(8):
                    for h01 in range(2):
                        ph = h01 * 64
                        nc.tensor.transpose(
                            pss[h01][:, b * 64 : (b + 1) * 64],
                            vc[ph : ph + 64, b * 128 : (b + 1) * 128],
                            identv_sb[ph : ph + 64, :],
                        )
                for h01 in range(2):
                    nc.vector.tensor_copy(
                        vnat[:, 2 * c + h01, :, 0:D],
                        pss[h01].rearrange("p (a b) -> p a b", b=64),
                    )

        # ====== gate projection, natural layout ============================
        for qc in range(8):
            gps = qkp.tile([128, 512], F32, name="gps", tag="qk")
            for ks in range(4):
                nc.tensor.matmul(
                    gps[:],
                    qdT[:, ks, qc * 128 : (qc + 1) * 128],
                    w_sb["g"][:, ks, :],
                    start=(ks == 0),
                    stop=(False if ONESMM else ks == 3),
                )
            if ONESMM:
                nc.tensor.matmul(
                    gps[:], ones_sb[:], bgn_sb[:], start=False, stop=True,
                )
            nc.scalar.activation(gN[:, qc, :], gps[:], FP.Sigmoid)

        # ====== attention ==================================================
        def av_block(hp, qcs):
            eTv = eT[hp % 2]
            for qc in qcs:
                av = avp.tile([128, 2, 66], F32, name="av", tag="av")
                for h01 in range(2):
                    for kc in range(8):
                        nc.tensor.matmul(
                            av[:, h01, 0 : D + 1],
                            eTv[h01][:, kc, qc * 128 : (qc + 1) * 128],
                            vnat[:, 2 * hp + h01, kc, :],
                            start=(kc == 0),
                            stop=(kc == 7),
                        )
                rec = recp.tile([128, 2], F32, name="rec", tag="rec")
                for h01 in range(2):
                    nc.vector.reciprocal_approx_fast(
                        rec[:, h01 : h01 + 1], av[:, h01, D : D + 1]
                    )
                for h01 in range(2):
                    nc.vector.tensor_scalar_mul(
                        oN[:, qc, 2 * hp + h01, :],
                        av[:, h01, 0:D],
                        rec[:, h01 : h01 + 1],
                    )
                (nc.gpsimd if GPS else nc.vector).tensor_tensor(
                    oN[:, qc, 2 * hp : 2 * hp + 2, :],
                    oN[:, qc, 2 * hp : 2 * hp + 2, :],
                    gN[:, qc, 2 * hp * 64 : (2 * hp + 2) * 64].rearrange(
                        "p (b c) -> p b c", b=2
                    ),
                    MULT,
                )

        for hp in range(4):
            eTp = eT[hp % 2]
            for j in range(4):
                ebs = []
                for h01 in range(2):
                    eb = ebp.tile([128, 2, L], BF16D, name="ebst", tag="ebst")
                    nc.sync.dma_start(
                        eb[:],
                        ebias[
                            hp, 2 * j : 2 * j + 2, :,
                            h01 * 1024 : (h01 + 1) * 1024,
                        ].rearrange("kc p q -> p kc q"),
                    )
                    ebs.append(eb)
                for kk in range(2):
                    kc = 2 * j + kk
                    for h01 in range(2):
                        ph = h01 * 64
                        qk = qkp.tile([128, L], F32, name="qk", tag="qk")
                        for q2 in range(2):
                            nc.tensor.matmul(
                                qk[:, q2 * 512 : (q2 + 1) * 512],
                                kc_t[hp][ph : ph + 64, kc * 128 : (kc + 1) * 128],
                                qc_t[hp][ph : ph + 64, q2 * 512 : (q2 + 1) * 512],
                                start=True,
                                stop=True,
                            )
                        nc.scalar.activation(
                            eTp[h01][:, kc, :], qk[:], FP.Exp,
                            bias=maskneg_sb[:, kc : kc + 1], scale=SCALE,
                        )
                for h01 in range(2):
                    nc.vector.tensor_tensor(
                        eTp[h01][:, 2 * j : 2 * j + 2, :],
                        eTp[h01][:, 2 * j : 2 * j + 2, :],
                        ebs[h01][:],
                        MULT,
                    )
            av_block(hp, range(8))

        # ====== epilogue: transpose og, out-projection =====================
        for qc in range(8):
            tp = qkp.tile([128, 4, 128], BF16D, name="ogtp", tag="qk")
            for vdc in range(4):
                nc.tensor.transpose(
                    tp[:, vdc, :],
                    oN[:, qc, 2 * vdc : 2 * vdc + 2, :].rearrange(
                        "p a b -> p (a b)"
                    ),
                    idento_sb[:],
                )
            nc.vector.tensor_copy(ogT[:, :, qc * 128 : (qc + 1) * 128], tp[:])
        for qc in range(8):
            pso = avp.tile([128, 512], F32, name="ops", tag="av")
            for vdc in range(4):
                nc.tensor.matmul(
                    pso[:],
                    ogT[:, vdc, qc * 128 : (qc + 1) * 128],
                    wo_sb[:, vdc, :],
                    start=(vdc == 0),
                    stop=(vdc == 3),
                )
            ost = stg.tile([128, C], F32, name="ost", tag="ost")
            nc.vector.tensor_tensor(ost[:], pso[:], bob_sb[:], ADD)
            nc.sync.dma_start(out[qc], ost[:])


# ---------------------------------------------------------------------------
# host side
# ---------------------------------------------------------------------------
_NC = None


def _get_nc():
    global _NC
    if _NC is None:
        _NC = build()
    return _NC


def _chunked(w):
    """(512, N) -> (128, 4, N) with row r at [r % 128, r // 128]."""
    n = np.asarray(w).shape[1]
    return np.ascontiguousarray(
        np.asarray(w, np.float32).reshape(4, 128, n).transpose(1, 0, 2)
    ).astype(BF16)


def _ctrans(x):
    """(L, C) -> (128, 4, L) bf16 with channel r at [r % 128, r // 128]."""
    xT = np.asarray(x, np.float32).T  # (C, L)
    return np.ascontiguousarray(
        xT.reshape(4, 128, L).transpose(1, 0, 2)
    ).astype(BF16)


def _prep_inmaps(q_data, k_data, bias, k_mask, Wq, Wk, Wv, Wg, bg, Wo, bo,
                 qcw, qcb, kcw, kcb, vcw, vcb):
    f32 = np.float32
    # exp(bias)^T -> (4, 8, 128, 2048) bf16: [hp, kc, p, h01*1024 + q]
    ebT = np.exp(np.asarray(bias[0], f32)).transpose(0, 2, 1)  # (h, k, q)
    ebias = np.ascontiguousarray(
        ebT.reshape(4, 2, 8, 128, L).transpose(0, 2, 3, 1, 4).reshape(4, 8, 128, 2 * L)
    ).astype(BF16)

    wq_a, wk_a, wv_a, wg_a, wo_a = (_chunked(w) for w in (Wq, Wk, Wv, Wg, Wo))
    bgn = np.asarray(bg, f32).reshape(1, VD).astype(BF16)
    bob = np.ascontiguousarray(
        np.broadcast_to(np.asarray(bo, f32).reshape(1, C), (128, C))
    ).astype(f32)

    identv = np.zeros((128, 64), f32)
    identv[np.arange(128), np.arange(128) % 64] = 1.0
    identv = identv.astype(BF16)
    idento = np.eye(128, dtype=f32).astype(BF16)

    dd = np.arange(128) % 64

    def taps(w3, b1, use_real):
        cwc = np.zeros((128, 4), f32)
        if use_real:
            cwc[:, 0:3] = np.asarray(w3, f32)[dd, 0, :]
            cwc[:, 3] = np.asarray(b1, f32)[dd]
        else:
            cwc[:, 1] = 1.0
        return cwc

    in_maps = []
    for s in range(NCORES):
        real = s >= 4
        cwm = np.concatenate(
            [taps(qcw, qcb, real), taps(kcw, kcb, real), taps(vcw, vcb, real)],
            axis=1,
        ).astype(f32)
        mk = np.asarray(k_mask[0, s], np.int32).reshape(8, 128).T  # (128, 8)
        maskneg = np.where(mk != 0, 0.0, MASK_NEG).astype(f32)
        in_maps.append(
            {
                "qd": _ctrans(q_data[0, s]),
                "kd": _ctrans(k_data[0, s]),
                "ebias": ebias,
                "maskneg": maskneg,
                "wq": wq_a, "wk": wk_a, "wv": wv_a, "wg": wg_a, "wo": wo_a,
                "convw": cwm,
                "bgn": bgn,
                "bob": bob,
                "identv": identv,
                "idento": idento,
            }
        )
    return in_maps


def run(in_maps, trace=False):
    nc = _get_nc()
    return run_bass_kernel_spmd(
        nc, in_maps, core_ids=list(range(NCORES)), trace=trace
    )


def kernel(**inputs):
    in_maps = _prep_inmaps(**inputs)
    res = run(in_maps)
    outp = np.empty((1, NCORES, L, C), np.float32)
    for s in range(NCORES):
        outp[0, s] = res.results[s]["out"].reshape(L, C)
    return outp


# revision 18
# speedup vs baseline: 1.2095x; 1.0599x over previous
"""Trainium2 Bass kernel for gated inception-conv attention (8 cores, seq-parallel).

Shapes (hardcoded): q_data/k_data (1,8,1024,512) f32, bias (1,8,1024,1024) f32,
k_mask (1,8,1024) i32, Wq/Wk/Wv/Wg (512,512), bg (512), Wo (512,512), bo (512),
qcw/kcw/vcw (64,1,3), qcb/kcb/vcb (64).  Output (1,8,1024,512) f32.

Strategy (v3): one sequence per NeuronCore, all-bf16 compute, f32 PSUM.
 - Conv phase as in v1: projections on PE (psum -> padded sbuf via ACT),
   depthwise conv as 5 shifted DVE ops, v transposed to natural layout on PE.
 - Scores transposed [kpos, q]; exp on ACT (k-mask via bias port, D^-0.5 via
   scale port); exp(bias)^T multiplied in on GpSimd.
 - AV flipped vs v1: score chunks stationary [128kpos x 128q], v natural
   [128kpos x 65] (ones column -> softmax denominator) is the moving operand;
   output lands NATURAL [q, d], so normalization is a per-partition
   tensor_scalar riding the psum-drain copy (reciprocal_approx_fast).
 - Gate computed in natural layout (bg via a K=1 ones-row matmul), gating on
   GpSimd; PE transposes og for the final projection; out = natural [q, c]
   (+bo via DVE add of a host-broadcast tile), DMA'd per q-chunk.
"""

import os
import sys

sys.path.insert(0, "/opt/trn_rl_repo")

import numpy as np
import ml_dtypes

import concourse.bass as bass
import concourse.mybir as mybir
from concourse import bacc, tile
from concourse.bass_utils import run_bass_kernel_spmd

BF16 = ml_dtypes.bfloat16
F32 = mybir.dt.float32
BF16D = mybir.dt.bfloat16
FP = mybir.ActivationFunctionType
MULT = mybir.AluOpType.mult
ADD = mybir.AluOpType.add

H, D, L, C = 8, 64, 1024, 512
KD = VD = 512
NCORES = 8
MASK_NEG = -100000.0
SCALE = 0.125  # D**-0.5, applied at the exp

ONESMM = os.environ.get("K_ONESMM", "1") == "1"
GPS = os.environ.get("K_GPS", "1") == "1"


def build():
    nc = bacc.Bacc(
        "TRN2",
        target_bir_lowering=False,
        debug=False,
        enable_asserts=False,
    )

    # ---- DRAM I/O (host pre-laid-out) ----
    qd = nc.dram_tensor("qd", [128, 4, L], BF16D, kind="ExternalInput").ap()
    kd = nc.dram_tensor("kd", [128, 4, L], BF16D, kind="ExternalInput").ap()
    # exp(bias)^T, bf16, fused head pairs: [hp, kc, p, h01*1024 + q]
    ebias = nc.dram_tensor("ebias", [4, 8, 128, 2048], BF16D, kind="ExternalInput").ap()
    maskneg = nc.dram_tensor("maskneg", [128, 8], F32, kind="ExternalInput").ap()
    wq = nc.dram_tensor("wq", [128, 4, KD], BF16D, kind="ExternalInput").ap()
    wk = nc.dram_tensor("wk", [128, 4, KD], BF16D, kind="ExternalInput").ap()
    wv = nc.dram_tensor("wv", [128, 4, VD], BF16D, kind="ExternalInput").ap()
    wg = nc.dram_tensor("wg", [128, 4, VD], BF16D, kind="ExternalInput").ap()
    wo = nc.dram_tensor("wo", [128, 4, C], BF16D, kind="ExternalInput").ap()
    convw = nc.dram_tensor("convw", [128, 12], F32, kind="ExternalInput").ap()
    bgn = nc.dram_tensor("bgn", [1, VD], BF16D, kind="ExternalInput").ap()
    bob = nc.dram_tensor("bob", [128, C], F32, kind="ExternalInput").ap()
    identv = nc.dram_tensor("identv", [128, 64], BF16D, kind="ExternalInput").ap()
    idento = nc.dram_tensor("idento", [128, 128], BF16D, kind="ExternalInput").ap()
    out = nc.dram_tensor("out", [8, 128, C], F32, kind="ExternalOutput").ap()

    with tile.TileContext(nc) as tc, nc.allow_low_precision(
        reason="bf16 compute; rel-err budget 2e-2"
    ):
        _body(tc, locals())
    nc.compile()
    return nc


def _body(tc, t):
    nc = tc.nc
    qd, kd, ebias, maskneg = t["qd"], t["kd"], t["ebias"], t["maskneg"]
    wq, wk, wv, wg, wo = t["wq"], t["wk"], t["wv"], t["wg"], t["wo"]
    convw, bgn, bob, identv, idento, out = (
        t["convw"], t["bgn"], t["bob"], t["identv"], t["idento"], t["out"],
    )

    with tc.tile_pool(name="const", bufs=1) as const, \
         tc.tile_pool(name="big", bufs=1) as big, \
         tc.tile_pool(name="ebp", bufs=6) as ebp, \
         tc.tile_pool(name="inw", bufs=1) as inw, \
         tc.tile_pool(name="recp", bufs=2) as recp, \
         tc.tile_pool(name="stage", bufs=2) as stg, \
         tc.tile_pool(name="qkps", bufs=3, space="PSUM") as qkp, \
         tc.tile_pool(name="avps", bufs=2, space="PSUM") as avp:

        # ---- inputs needed first: activations + projection weights ----
        qdT = inw.tile([128, 4, L], BF16D, name="qdT", tag="qdT")
        for ks in range(4):
            nc.sync.dma_start(qdT[:, ks, :], qd[:, ks, :])
        w_sb = {}
        for nm, wd in (("q", wq), ("k", wk), ("v", wv), ("g", wg)):
            w_sb[nm] = inw.tile([128, 4, 512], BF16D, name=f"w{nm}", tag=f"w{nm}")
        nc.sync.dma_start(w_sb["q"][:], wq)
        kdT = inw.tile([128, 4, L], BF16D, name="kdT", tag="kdT")
        for ks in range(4):
            nc.sync.dma_start(kdT[:, ks, :], kd[:, ks, :])
        nc.sync.dma_start(w_sb["k"][:], wk)
        nc.sync.dma_start(w_sb["v"][:], wv)
        nc.sync.dma_start(w_sb["g"][:], wg)

        # ---- other constants ----
        convw_sb = const.tile([128, 12], F32, name="convw", tag="convw")
        nc.sync.dma_start(convw_sb[:], convw)
        identv_sb = const.tile([128, 64], BF16D, name="identv", tag="identv")
        nc.sync.dma_start(identv_sb[:], identv)
        idento_sb = const.tile([128, 128], BF16D, name="idento", tag="idento")
        nc.sync.dma_start(idento_sb[:], idento)
        maskneg_sb = const.tile([128, 8], F32, name="maskneg", tag="maskneg")
        nc.sync.dma_start(maskneg_sb[:], maskneg)
        bgn_sb = const.tile([1, VD], BF16D, name="bgn", tag="bgn")
        nc.sync.dma_start(bgn_sb[:], bgn)
        bob_sb = const.tile([128, C], F32, name="bob", tag="bob")
        nc.sync.dma_start(bob_sb[:], bob)
        wo_sb = const.tile([128, 4, C], BF16D, name="wo", tag="wo")
        nc.sync.dma_start(wo_sb[:], wo)
        ones_sb = const.tile([1, 128], BF16D, name="ones", tag="ones")
        nc.vector.memset(ones_sb[:], 1.0)

        # ---- persistent big tensors ----
        qc_t = [big.tile([128, L], BF16D, name=f"qc{c}", tag=f"qc{c}") for c in range(4)]
        kc_t = [big.tile([128, L], BF16D, name=f"kc{c}", tag=f"kc{c}") for c in range(4)]
        vnat = big.tile([128, H, 8, D + 1], BF16D, name="vnat", tag="vnat")
        nc.vector.memset(vnat[:, :, :, D : D + 1], 1.0)
        gN = big.tile([128, 8, VD], BF16D, name="gN", tag="gN")
        oN = big.tile([128, 8, H, D], BF16D, name="oN", tag="oN")
        eT = [
            [
                big.tile([128, 8, L], BF16D, name=f"eT{par}{h01}",
                         tag=f"eT{par}{h01}")
                for h01 in range(2)
            ]
            for par in range(2)
        ]
        ogT = big.tile([128, 4, L], BF16D, name="ogT", tag="ogT")

        # ====== conv phase (v1 structure): projections -> conv -> v-T ======
        rhsT = {"q": qdT, "k": kdT, "v": kdT}
        cw = {"q": 0, "k": 4, "v": 8}
        with tc.tile_pool(name="pad", bufs=2) as padp, \
             tc.tile_pool(name="cvtmp", bufs=2) as cvp, \
             tc.tile_pool(name="vc", bufs=2) as vcp:
            for c in range(4):
                pads = {}
                for nm in ("q", "k", "v"):
                    pads[nm] = padp.tile(
                        [128, L + 2], BF16D, name=f"pad{nm}", tag=f"pad{nm}"
                    )
                    nc.vector.memset(pads[nm][:, 0:1], 0.0)
                    nc.vector.memset(pads[nm][:, L + 1 : L + 2], 0.0)
                    for q2 in range(2):
                        ps = qkp.tile([128, 512], F32, name="projps", tag="qk")
                        for ks in range(4):
                            nc.tensor.matmul(
                                ps[:],
                                w_sb[nm][:, ks, c * 128 : (c + 1) * 128],
                                rhsT[nm][:, ks, q2 * 512 : (q2 + 1) * 512],
                                start=(ks == 0),
                                stop=(ks == 3),
                            )
                        nc.scalar.copy(
                            pads[nm][:, 1 + q2 * 512 : 1 + (q2 + 1) * 512], ps[:]
                        )
                vc = vcp.tile([128, L], BF16D, name="vc", tag="vc")
                dstc = {"q": qc_t[c], "k": kc_t[c], "v": vc}
                for nm in ("q", "k", "v"):
                    base = cw[nm]
                    w0 = convw_sb[:, base : base + 1]
                    w1 = convw_sb[:, base + 1 : base + 2]
                    w2 = convw_sb[:, base + 2 : base + 3]
                    bb = convw_sb[:, base + 3 : base + 4]
                    x = pads[nm]
                    y = dstc[nm]
                    nc.vector.tensor_scalar(y[:], x[:, 1 : L + 1], w1, bb, MULT, ADD)
                    tm = cvp.tile([128, L], BF16D, name="cvtmp", tag="cvtmp")
                    nc.vector.tensor_scalar_mul(tm[:], x[:, 0:L], w0)
                    nc.vector.tensor_tensor(y[:], y[:], tm[:], ADD)
                    tm2 = cvp.tile([128, L], BF16D, name="cvtmp2", tag="cvtmp")
                    nc.vector.tensor_scalar_mul(tm2[:], x[:, 2 : L + 2], w2)
                    nc.vector.tensor_tensor(y[:], y[:], tm2[:], ADD)
                # v -> natural layout, heads 2c/2c+1 row-packed on the PE
                pss = [
                    avp.tile([128, 512], BF16D, name="vtps", tag="av")
                    for _ in range(2)
                ]
                for b in range(8):
                    for h01 in range(2):
                        ph = h01 * 64
                        nc.tensor.transpose(
                            pss[h01][:, b * 64 : (b + 1) * 64],
                            vc[ph : ph + 64, b * 128 : (b + 1) * 128],
                            identv_sb[ph : ph + 64, :],
                        )
                for h01 in range(2):
                    nc.vector.tensor_copy(
                        vnat[:, 2 * c + h01, :, 0:D],
                        pss[h01].rearrange("p (a b) -> p a b", b=64),
                    )

        # ====== gate projection, natural layout ============================
        for qc in range(8):
            gps = qkp.tile([128, 512], F32, name="gps", tag="qk")
            for ks in range(4):
                nc.tensor.matmul(
                    gps[:],
                    qdT[:, ks, qc * 128 : (qc + 1) * 128],
                    w_sb["g"][:, ks, :],
                    start=(ks == 0),
                    stop=(False if ONESMM else ks == 3),
                )
            if ONESMM:
                nc.tensor.matmul(
                    gps[:], ones_sb[:], bgn_sb[:], start=False, stop=True,
                )
            nc.scalar.activation(gN[:, qc, :], gps[:], FP.Sigmoid)

        # ====== attention ==================================================
        def av_block(hp, qcs):
            eTv = eT[hp % 2]
            for qc in qcs:
                av = avp.tile([128, 2, 66], F32, name="av", tag="av")
                for h01 in range(2):
                    for kc in range(8):
                        nc.tensor.matmul(
                            av[:, h01, 0 : D + 1],
                            eTv[h01][:, kc, qc * 128 : (qc + 1) * 128],
                            vnat[:, 2 * hp + h01, kc, :],
                            start=(kc == 0),
                            stop=(kc == 7),
                        )
                rec = recp.tile([128, 2], F32, name="rec", tag="rec")
                for h01 in range(2):
                    nc.vector.reciprocal_approx_fast(
                        rec[:, h01 : h01 + 1], av[:, h01, D : D + 1]
                    )
                for h01 in range(2):
                    nc.vector.tensor_scalar_mul(
                        oN[:, qc, 2 * hp + h01, :],
                        av[:, h01, 0:D],
                        rec[:, h01 : h01 + 1],
                    )
                (nc.gpsimd if GPS else nc.vector).tensor_tensor(
                    oN[:, qc, 2 * hp : 2 * hp + 2, :],
                    oN[:, qc, 2 * hp : 2 * hp + 2, :],
                    gN[:, qc, 2 * hp * 64 : (2 * hp + 2) * 64].rearrange(
                        "p (b c) -> p b c", b=2
                    ),
                    MULT,
                )

        for hp in range(4):
            eTp = eT[hp % 2]
            for j in range(4):
                ebs = []
                for h01 in range(2):
                    eb = ebp.tile([128, 2, L], BF16D, name="ebst", tag="ebst")
                    nc.sync.dma_start(
                        eb[:],
                        ebias[
                            hp, 2 * j : 2 * j + 2, :,
                            h01 * 1024 : (h01 + 1) * 1024,
                        ].rearrange("kc p q -> p kc q"),
                    )
                    ebs.append(eb)
                for kk in range(2):
                    kc = 2 * j + kk
                    for h01 in range(2):
                        ph = h01 * 64
                        qk = qkp.tile([128, L], F32, name="qk", tag="qk")
                        for q2 in range(2):
                            nc.tensor.matmul(
                                qk[:, q2 * 512 : (q2 + 1) * 512],
                                kc_t[hp][ph : ph + 64, kc * 128 : (kc + 1) * 128],
                                qc_t[hp][ph : ph + 64, q2 * 512 : (q2 + 1) * 512],
                                start=True,
                                stop=True,
                            )
                        nc.scalar.activation(
                            eTp[h01][:, kc, :], qk[:], FP.Exp,
                            bias=maskneg_sb[:, kc : kc + 1], scale=SCALE,
                        )
                for h01 in range(2):
                    nc.vector.tensor_tensor(
                        eTp[h01][:, 2 * j : 2 * j + 2, :],
                        eTp[h01][:, 2 * j : 2 * j + 2, :],
                        ebs[h01][:],
                        MULT,
                    )
            av_block(hp, range(8))

        # ====== epilogue: transpose og, out-projection =====================
        for qc in range(8):
            tp = qkp.tile([128, 4, 128], BF16D, name="ogtp", tag="qk")
            for vdc in range(4):
                nc.tensor.transpose(
                    tp[:, vdc, :],
                    oN[:, qc, 2 * vdc : 2 * vdc + 2, :].rearrange(
                        "p a b -> p (a b)"
                    ),
                    idento_sb[:],
                )
            nc.vector.tensor_copy(ogT[:, :, qc * 128 : (qc + 1) * 128], tp[:])
        for qc in range(8):
            pso = avp.tile([128, 512], F32, name="ops", tag="av")
            for vdc in range(4):
                nc.tensor.matmul(
                    pso[:],
                    ogT[:, vdc, qc * 128 : (qc + 1) * 128],
                    wo_sb[:, vdc, :],
                    start=(vdc == 0),
                    stop=(vdc == 3),
                )
            ost = stg.tile([128, C], F32, name="ost", tag="ost")
            nc.vector.tensor_tensor(ost[:], pso[:], bob_sb[:], ADD)
            nc.sync.dma_start(out[qc], ost[:])


# ---------------------------------------------------------------------------
# host side
# ---------------------------------------------------------------------------
_NC = None


def _get_nc():
    global _NC
    if _NC is None:
        _NC = build()
    return _NC


def _chunked(w):
    """(512, N) -> (128, 4, N) with row r at [r % 128, r // 128]."""
    n = np.asarray(w).shape[1]
    return np.ascontiguousarray(
        np.asarray(w, np.float32).reshape(4, 128, n).transpose(1, 0, 2)
    ).astype(BF16)


def _ctrans(x):
    """(L, C) -> (128, 4, L) bf16 with channel r at [r % 128, r // 128]."""
    xT = np.asarray(x, np.float32).T  # (C, L)
    return np.ascontiguousarray(
        xT.reshape(4, 128, L).transpose(1, 0, 2)
    ).astype(BF16)


def _prep_inmaps(q_data, k_data, bias, k_mask, Wq, Wk, Wv, Wg, bg, Wo, bo,
                 qcw, qcb, kcw, kcb, vcw, vcb):
    f32 = np.float32
    # exp(bias)^T -> (4, 8, 128, 2048) bf16: [hp, kc, p, h01*1024 + q]
    ebT = np.exp(np.asarray(bias[0], f32)).transpose(0, 2, 1)  # (h, k, q)
    ebias = np.ascontiguousarray(
        ebT.reshape(4, 2, 8, 128, L).transpose(0, 2, 3, 1, 4).reshape(4, 8, 128, 2 * L)
    ).astype(BF16)

    wq_a, wk_a, wv_a, wg_a, wo_a = (_chunked(w) for w in (Wq, Wk, Wv, Wg, Wo))
    bgn = np.asarray(bg, f32).reshape(1, VD).astype(BF16)
    bob = np.ascontiguousarray(
        np.broadcast_to(np.asarray(bo, f32).reshape(1, C), (128, C))
    ).astype(f32)

    identv = np.zeros((128, 64), f32)
    identv[np.arange(128), np.arange(128) % 64] = 1.0
    identv = identv.astype(BF16)
    idento = np.eye(128, dtype=f32).astype(BF16)

    dd = np.arange(128) % 64

    def taps(w3, b1, use_real):
        cwc = np.zeros((128, 4), f32)
        if use_real:
            cwc[:, 0:3] = np.asarray(w3, f32)[dd, 0, :]
            cwc[:, 3] = np.asarray(b1, f32)[dd]
        else:
            cwc[:, 1] = 1.0
        return cwc

    in_maps = []
    for s in range(NCORES):
        real = s >= 4
        cwm = np.concatenate(
            [taps(qcw, qcb, real), taps(kcw, kcb, real), taps(vcw, vcb, real)],
            axis=1,
        ).astype(f32)
        mk = np.asarray(k_mask[0, s], np.int32).reshape(8, 128).T  # (128, 8)
        maskneg = np.where(mk != 0, 0.0, MASK_NEG).astype(f32)
        in_maps.append(
            {
                "qd": _ctrans(q_data[0, s]),
                "kd": _ctrans(k_data[0, s]),
                "ebias": ebias,
                "maskneg": maskneg,
                "wq": wq_a, "wk": wk_a, "wv": wv_a, "wg": wg_a, "wo": wo_a,
                "convw": cwm,
                "bgn": bgn,
                "bob": bob,
                "identv": identv,
                "idento": idento,
            }
        )
    return in_maps


def run(in_maps, trace=False):
    nc = _get_nc()
    return run_bass_kernel_spmd(
        nc, in_maps, core_ids=list(range(NCORES)), trace=trace
    )


def kernel(**inputs):
    in_maps = _prep_inmaps(**inputs)
    res = run(in_maps)
    outp = np.empty((1, NCORES, L, C), np.float32)
    for s in range(NCORES):
        outp[0, s] = res.results[s]["out"].reshape(L, C)
    return outp
